# revision 33
# baseline (speedup 1.0000x reference)
"""Causal self-attention (B=4, T=2048, C=1024, H=16) on 8 TRN2 NeuronCores.

Sharding: batch x head-group. Core c owns batch b=c//2 and heads
8*(c%2)..8*(c%2)+7 (4 head-pairs). Host sums the two partial [T, C]
outputs per batch and adds b_proj.

Schedule: the ScalarE exp stream (~172us) is the slow stage of the
attention inner loop, so all non-attention matmuls (qkv projections,
v production, output projection) are chopped into single-matmul
"filler" steps and drained into the PE queue inside every kc2 step.
Groups run g-major (g outer, hp inner) so the output projection of
group g unlocks early and spreads across the run. The AV pass trails
the exp by a full kc2 step (depth-2 pipeline) so it never stalls on
ACT. Diagonal chunks are triangularly tightened (fully-masked 128-col
blocks are skipped in scores/AV/exp) and the remaining per-chunk
triangle is zeroed post-exp via gpsimd affine_select, off both the
PE and DVE critical chains.

Per core (all matmuls bf16, fp32 PSUM):
  - q,k projections dim-major (qT/kT [128, T], head-pair h0/h1 on
    partition halves); v produced key-major via x-stationary matmuls
    (no PE transposes), all 4 head-pairs per pass (N=512 moving),
    evacuated (+bias) straight into the AV operand layout.
  - Scores per head-pair: sT = kT.T @ qT with the two heads on PE row
    groups 0-63/64-127 (row-tiled, streams concurrently).
  - AV is p-stationary: per 128-query block the exp'd scores load as
    weights (128 cols, fast weight load) and only the 65 v+ones
    columns stream, halving AV moving columns; outputs land
    query-major [q, qb, d] with the ones-column Z sums at d=64.
    PSUM gotcha: matmul start=True clears has_written for the whole
    bank, so only the first matmul into each AV tile carries it.
  - Normalize is a per-partition reciprocal + multiply (no partition
    shifts); one tiled transpose-DMA per (hp, g) returns y to
    dim-major yu[d, hp, q] (dma_start_transpose tiles columns by 128:
    out[dp, a, q] = in[q, 128a + dp]).
  - Projection contracts all 128 dims of each head-pair at once
    (K=128), accumulating the 4 head-pairs into one PSUM tile.
"""

import numpy as np

import concourse.bass as bass
import concourse.mybir as mybir
import concourse.tile as tile
from concourse import bacc
from concourse.bass_utils import run_bass_kernel_spmd

B, T, C, H = 4, 2048, 1024, 16
NCORES = 8
HP = 4                      # head-pairs per core
QG = T // 512               # 4 query groups
CPB = T // 128              # 16 key chunks
f32 = mybir.dt.float32
bf16 = mybir.dt.bfloat16
EXP = mybir.ActivationFunctionType.Exp

TRACE = False
DEBUG_YU = False
TRACE_KWARGS = {}
LAST_RESULT = None
_NC_CACHE = None


def _emit(tc, x16, w16, wv16, b_qk, bv, wp16, out):
    nc = tc.nc
    with (
        tc.tile_pool(name="const", bufs=1) as constp,
        tc.tile_pool(name="persist", bufs=1) as persp,
        tc.tile_pool(name="stage", bufs=4) as stp,
        tc.tile_pool(name="pt", bufs=4) as ptp,
        tc.tile_pool(name="ost", bufs=4) as ostp,
        tc.tile_pool(name="ps_s", bufs=2, space="PSUM") as ps_s,
        tc.tile_pool(name="ps_o", bufs=2, space="PSUM") as ps_o,
        tc.tile_pool(name="aux", bufs=2, space="PSUM") as auxp,
    ):
        bqksb = constp.tile([128, 2 * HP], f32)
        bvsb = constp.tile([1, 512], f32)
        x16sb = constp.tile([128, 8, T], bf16)
        w16sb = constp.tile([128, 8, 1024], bf16)
        wv16sb = constp.tile([128, 8, 512], bf16)
        # interleave per contraction chunk: first qk + v matmuls can
        # issue as soon as the first chunks land (DMA issue itself is
        # ~0.6us per descriptor, so order = dependency order)
        # x16 issues on the ScalarE DGE queue (idle until the first
        # exp ~15us in) so the two DMA rings issue in parallel
        for kc in range(8):
            nc.sync.dma_start(w16sb[:, kc, :], w16[:, kc, :])
            nc.scalar.dma_start(x16sb[:, kc, 0:1024], x16[:, kc, 0:1024])
            nc.sync.dma_start(wv16sb[:, kc, :], wv16[:, kc, :])
            if kc == 1:
                nc.sync.dma_start(bqksb[:], b_qk)
                nc.sync.dma_start(bvsb[:], bv)
        for kc in range(8):
            nc.scalar.dma_start(x16sb[:, kc, 1024:2048], x16[:, kc, 1024:2048])
        wp16sb = constp.tile([128, HP, C], bf16)
        nc.scalar.dma_start(wp16sb[:], wp16)
        c1 = constp.tile([128, CPB], bf16)
        nc.gpsimd.memset(c1[:], 1.0)
        # v bias broadcast over key partitions: bvb[:, hp, :] = b_v[hp dims]
        bvb = constp.tile([128, HP, 128], f32)
        for hp in range(HP):
            nc.gpsimd.partition_broadcast(
                bvb[:, hp, :], bvsb[0:1, 128 * hp : 128 * (hp + 1)], channels=128
            )

        # additive causal triangle, applied to scores PSUM before exp:
        # 0 where col >= partition (keep), -1e5 where masked
        negtri = constp.tile([128, 128], f32)
        nc.gpsimd.memset(negtri[:], 0.0)
        nc.gpsimd.affine_select(
            out=negtri[:], in_=negtri[:], compare_op=mybir.AluOpType.is_ge,
            fill=-1.0e5, base=0, channel_multiplier=-1, pattern=[[1, 128]],
        )

        qT = persp.tile([128, HP, T], bf16, name="qT")
        kT = persp.tile([128, HP, T], bf16, name="kT")
        vaug = persp.tile([128, HP, CPB, 130], bf16, name="vaug")
        yu = persp.tile([128, HP, T], bf16, name="yu")
        _DBG = {}
        if DEBUG_YU:
            _DBG["zdbg"] = persp.tile([128, HP, QG, 2, 4], f32, name="zdbg")

        for hp in range(HP):
            nc.vector.tensor_copy(vaug[:, hp, :, 64:65], c1[:].unsqueeze(2))
            nc.vector.tensor_copy(vaug[:, hp, :, 129:130], c1[:].unsqueeze(2))

        # ---- filler units: generators yielding (cols, emit_fn) ----

        def gen_qk(hp, np_):
            for t in range(2):  # 0=q, 1=k
                dst = qT if t == 0 else kT
                msl = slice(256 * hp + 128 * t, 256 * hp + 128 * (t + 1))
                for i in range(2):
                    n = 2 * np_ + i
                    box = {}

                    def alloc(box=box, hp=hp, np_=np_, t=t, i=i):
                        box["ps"] = auxp.tile(
                            [128, 512], f32, name=f"qk_{hp}_{np_}_{t}_{i}", tag="aux"
                        )

                    for kc in range(8):
                        def mm(kc=kc, n=n, msl=msl, box=box, alloc=alloc):
                            if kc == 0:
                                alloc()
                            nc.tensor.matmul(
                                box["ps"][:],
                                w16sb[:, kc, msl],
                                x16sb[:, kc, 512 * n : 512 * (n + 1)],
                                start=(kc == 0),
                                stop=(kc == 7),
                            )
                        yield (512, mm)

                    def evac(box=box, dst=dst, n=n, hp=hp, t=t):
                        nc.vector.tensor_scalar_add(
                            dst[:, hp, 512 * n : 512 * (n + 1)],
                            box["ps"][:],
                            bqksb[:, 2 * hp + t : 2 * hp + t + 1],
                        )
                    yield (0, evac)

        def gen_v(kbq):
            for kb in range(4 * kbq, 4 * kbq + 4):
                box = {}

                def alloc(box=box, kb=kb):
                    box["ps"] = auxp.tile([128, 512], f32, name=f"vt_{kb}", tag="aux")

                for kc in range(8):
                    def mm(kc=kc, kb=kb, box=box, alloc=alloc):
                        if kc == 0:
                            alloc()
                        nc.tensor.matmul(
                            box["ps"][:],
                            x16sb[:, kc, 128 * kb : 128 * (kb + 1)],
                            wv16sb[:, kc, :],
                            start=(kc == 0),
                            stop=(kc == 7),
                        )
                    yield (512, mm)

                def evac(box=box, kb=kb):
                    for hp in range(HP):
                        dst = vaug[:, hp, kb, 0:130].rearrange(
                            "p (two x) -> p two x", x=65
                        )[:, :, 0:64]
                        nc.vector.tensor_tensor(
                            dst,
                            box["ps"][:, 128 * hp : 128 * (hp + 1)].rearrange(
                                "p (two d) -> p two d", d=64
                            ),
                            bvb[:, hp, :].rearrange("p (two d) -> p two d", d=64),
                            op=mybir.AluOpType.add,
                        )
                yield (0, evac)

        def gen_proj(rt):
            rsl = slice(128 * rt, 128 * (rt + 1))
            for jn in range(2):
                nsl = slice(512 * jn, 512 * (jn + 1))
                box = {}

                def alloc(box=box, rt=rt, jn=jn):
                    box["ps"] = auxp.tile(
                        [128, 512], f32, name=f"pp_{rt}_{jn}", tag="aux"
                    )

                for hp in range(HP):
                    def mm(hp=hp, rsl=rsl, nsl=nsl, box=box, alloc=alloc):
                        if hp == 0:
                            alloc()
                        nc.tensor.matmul(
                            box["ps"][:],
                            yu[:, hp, rsl],
                            wp16sb[:, hp, nsl],
                            start=(hp == 0),
                            stop=(hp == HP - 1),
                        )
                    yield (512, mm)

                def evac(box=box, rt=rt, jn=jn, rsl=rsl, nsl=nsl):
                    ost = ostp.tile([128, 512], f32, name=f"ost_{rt}_{jn}", tag="ost")
                    nc.vector.tensor_copy(ost[:], box["ps"][:])
                    nc.sync.dma_start(out[rsl, nsl], ost[:])
                yield (0, evac)

        # dependency-ordered unit queue; proj units are appended as
        # their inputs (yu rows for group g, all head-pairs) complete
        units = [
            ("qk00", gen_qk(0, 0)), ("v0", gen_v(0)),
            ("qk10", gen_qk(1, 0)), ("qk20", gen_qk(2, 0)), ("qk30", gen_qk(3, 0)),
            ("v1", gen_v(1)),
            ("qk01", gen_qk(0, 1)), ("qk11", gen_qk(1, 1)),
            ("v2", gen_v(2)),
            ("qk21", gen_qk(2, 1)), ("qk31", gen_qk(3, 1)),
            ("v3", gen_v(3)),
        ]
        done = set()
        cursor = [0]

        def fill(budget):
            while cursor[0] < len(units):
                name, g_ = units[cursor[0]]
                advanced = False
                for cols, fn in g_:
                    fn()
                    advanced = True
                    budget -= cols
                    if budget <= 0:
                        return
                if not advanced or True:
                    done.add(name)
                    cursor[0] += 1

        def require(name):
            while name not in done:
                assert cursor[0] < len(units), f"unit {name} not queued"
                uname, g_ = units[cursor[0]]
                for cols, fn in g_:
                    fn()
                done.add(uname)
                cursor[0] += 1

        # ---- attention ----

        def att(hp, g):
            nkc2 = 2 * g + 2
            gsl = slice(512 * g, 512 * (g + 1))
            # AV output is query-major: ots[h][q, qb, d] with the Z
            # (ones-column) sums at d=64, so normalization is a cheap
            # per-partition scalar multiply (no partition shifts)
            # padded to 128 so each query-block lands bank-aligned
            ots = [
                ps_o.tile([128, 4, 128], f32, name=f"ot_{hp}_{g}_{h}", tag="ot")
                for h in range(2)
            ]

            def off_of(kc):
                j = kc - 4 * g
                return 128 * j if j > 0 else 0

            def scores(kc2):
                sps = [
                    ps_s.tile([128, 1024], f32, name=f"sp_{hp}_{g}_{kc2}_{h}", tag="sp")
                    for h in range(2)
                ]
                for half in range(2):
                    kc = 2 * kc2 + half
                    off = off_of(kc)
                    for h in range(2):
                        hsl = slice(64 * h, 64 * h + 64)
                        nc.tensor.matmul(
                            sps[h][:, 512 * half + off : 512 * (half + 1)],
                            kT[hsl, hp, 128 * kc : 128 * (kc + 1)],
                            qT[hsl, hp, 512 * g + off : 512 * (g + 1)],
                            start=True,
                            stop=True,
                        )
                return sps

            def expmask(kc2, sps):
                # mask diagonal triangles additively in PSUM before exp:
                # the only consumer afterwards is the strict-FIFO ACT
                # exp, so no engine can observe unmasked scores
                for half in range(2):
                    kc = 2 * kc2 + half
                    j = kc - 4 * g
                    if j < 0:
                        continue
                    for h in range(2):
                        v_ = sps[h][:, 512 * half + 128 * j : 512 * half + 128 * (j + 1)]
                        nc.vector.tensor_tensor(
                            v_, v_, negtri[:], op=mybir.AluOpType.add
                        )
                pt = ptp.tile([128, 2, 1024], bf16, name=f"pt_{hp}_{g}_{kc2}", tag="pt")
                s0 = 512 * 0 + off_of(2 * kc2)  # exp span start (skip all-masked)
                for h in range(2):
                    nc.scalar.activation(
                        pt[:, h, s0:1024], sps[h][:, s0:1024], EXP, scale=0.125
                    )
                return pt

            def av(kc2, pt):
                # p-stationary: per 128-query block, load exp-scores as
                # weights (128 cols, FWL) and stream only the 65 v+ones
                # columns -- ~2x fewer moving columns than p-moving
                for h in range(2):
                    for half in range(2):
                        kc = 2 * kc2 + half
                        j = kc - 4 * g
                        for qb in range(max(j, 0), 4):
                            # start=True clears has_written for the WHOLE
                            # bank, so only the very first matmul into the
                            # tile may carry it; later first-writes rely on
                            # accumulate-mode overwriting cleared elements
                            nc.tensor.matmul(
                                ots[h][:, qb, 0:65],
                                pt[:, h, 512 * half + 128 * qb : 512 * half + 128 * (qb + 1)],
                                vaug[:, hp, kc, 65 * h : 65 * h + 65],
                                start=(kc == 0 and qb == 0),
                                stop=(kc == 4 * g + qb),
                                skip_group_check=True,
                            )

            # depth-1 software pipeline with filler drain between the
            # next scores and the exp-dependent av
            # depth-2: av(k-1) issues after scores(k+1), so its exp
            # finished a full step earlier and the PE never stalls on ACT
            cur = scores(0)
            pend = None
            for kc2 in range(nkc2):
                pt = expmask(kc2, cur)
                if kc2 + 1 < nkc2:
                    cur = scores(kc2 + 1)
                if pend is not None:
                    av(*pend)
                fill(2560)
                pend = (kc2, pt)
            av(*pend)

            # yu2[q, qb, 64h+d] = ots[h][q, qb, d] / Z; one tiled
            # transpose-DMA then yields yu[64h+d, hp, 128qb+q]
            # (dma_start_transpose: out[dp, a, q] = in[q, 128a + dp])
            yu2 = stp.tile([128, 4, 128], bf16, name=f"yu2_{hp}_{g}", tag="yu2")
            for h in range(2):
                rz = stp.tile([128, 4], f32, name=f"rz_{hp}_{g}_{h}", tag="rz")
                nc.vector.reciprocal_approx_fast(
                    rz[:], ots[h][:, :, 64:65].rearrange("p a one -> p (a one)")
                )
                if DEBUG_YU:
                    zd = _DBG["zdbg"]
                    nc.vector.tensor_copy(
                        zd[:, hp, g, h, :],
                        ots[h][:, :, 64:65].rearrange("p a one -> p (a one)"),
                    )
                nc.vector.tensor_tensor(
                    yu2[:, :, 64 * h : 64 * h + 64],
                    ots[h][:, :, 0:64],
                    rz[:].unsqueeze(2).broadcast_to([128, 4, 64]),
                    op=mybir.AluOpType.mult,
                )
            # scalar queue: skips the proj out-DMA backlog on sync,
            # shortening the normalize->proj chain at group ends
            nc.scalar.dma_start_transpose(
                yu[:, hp, gsl].rearrange("p (a b) -> p a b", b=128),
                yu2[:],
            )

        # ---- g-major schedule ----
        # zero the two score-PSUM ring slots once: exp spans include
        # columns the tightened scores never write; on a fresh device
        # those would otherwise be uninitialized (NaN risk)
        for i in range(2):
            spz = ps_s.tile([128, 1024], f32, name=f"spz_{i}", tag="sp")
            nc.vector.memset(spz[:], 0.0)
        require("qk00")
        require("v0")
        for g in range(QG):
            require(f"v{g}")
            for hp in range(HP):
                require(f"qk{hp}{g // 2}")
                att(hp, g)
            for rt in range(4 * g, 4 * (g + 1)):
                units.append((f"proj{rt}", gen_proj(rt)))
        # drain whatever fillers remain (late proj units)
        fill(1 << 30)
        if DEBUG_YU:
            ydbg = tc.nc.dram_tensor(
                "ydbg", [128, HP, T], bf16, kind="ExternalOutput"
            ).ap()
            nc.sync.dma_start(ydbg, yu[:])
            zdbg_d = tc.nc.dram_tensor(
                "zdbg", [128, HP, QG, 2, 4], f32, kind="ExternalOutput"
            ).ap()
            nc.sync.dma_start(zdbg_d, _DBG["zdbg"][:])


def build_nc():
    global _NC_CACHE
    if _NC_CACHE is not None:
        return _NC_CACHE
    nc = bacc.Bacc("TRN2", target_bir_lowering=False, debug=False)
    x16 = nc.dram_tensor("x16", [128, 8, T], bf16, kind="ExternalInput").ap()
    w16 = nc.dram_tensor("w16", [128, 8, 1024], bf16, kind="ExternalInput").ap()
    wv16 = nc.dram_tensor("wv16", [128, 8, 512], bf16, kind="ExternalInput").ap()
    b_qk = nc.dram_tensor("b_qk", [128, 2 * HP], f32, kind="ExternalInput").ap()
    bv = nc.dram_tensor("bv", [1, 512], f32, kind="ExternalInput").ap()
    wp16 = nc.dram_tensor("wp16", [128, HP, C], bf16, kind="ExternalInput").ap()
    out = nc.dram_tensor("out", [T, C], f32, kind="ExternalOutput").ap()
    with tile.TileContext(nc) as tc:
        _emit(tc, x16, w16, wv16, b_qk, bv, wp16, out)
    nc.compile()
    _NC_CACHE = nc
    return nc


def make_in_maps(x, w_attn, b_attn, w_proj):
    import ml_dtypes

    bf16np = ml_dtypes.bfloat16
    in_maps = []
    for core in range(NCORES):
        b, hg = divmod(core, 2)
        xb = np.asarray(x[b], np.float32)  # [T, C]
        # x16[p, kc, n] = xb[n, 128*kc + p]
        x16 = np.ascontiguousarray(
            xb.T.reshape(8, 128, T).transpose(1, 0, 2).astype(bf16np)
        )
        # w16[p, kc, hp*256 + t*128 + d] = w_attn[128*kc + p, tC + 512hg + 128hp + d]
        cols = []
        vcols = []
        for hp in range(HP):
            for toff in (0, C):
                c0 = toff + 512 * hg + 128 * hp
                cols.append(w_attn[:, c0 : c0 + 128])
            c0 = 2 * C + 512 * hg + 128 * hp
            vcols.append(w_attn[:, c0 : c0 + 128])
        wt = np.concatenate(cols, axis=1)  # [1024, 1024]
        w16 = np.ascontiguousarray(
            wt.reshape(8, 128, 1024).transpose(1, 0, 2).astype(bf16np)
        )
        wv = np.concatenate(vcols, axis=1)  # [1024, 512]
        wv16 = np.ascontiguousarray(
            wv.reshape(8, 128, 512).transpose(1, 0, 2).astype(bf16np)
        )
        b_qk = np.zeros((128, 2 * HP), np.float32)
        for hp in range(HP):
            for t, toff in enumerate((0, C)):
                c0 = toff + 512 * hg + 128 * hp
                b_qk[:, 2 * hp + t] = b_attn[c0 : c0 + 128]
        bv = np.zeros((1, 512), np.float32)
        for hp in range(HP):
            c0 = 2 * C + 512 * hg + 128 * hp
            bv[0, 128 * hp : 128 * hp + 128] = b_attn[c0 : c0 + 128]
        wp16 = np.ascontiguousarray(
            w_proj[512 * hg : 512 * hg + 512, :]
            .reshape(4, 128, C)
            .transpose(1, 0, 2)
            .astype(bf16np)
        )
        in_maps.append(
            {"x16": x16, "w16": w16, "wv16": wv16, "b_qk": b_qk, "bv": bv, "wp16": wp16}
        )
    return in_maps


def kernel(x, w_attn, b_attn, w_proj, b_proj):
    global LAST_RESULT
    x = np.asarray(x, dtype=np.float32)
    w_attn = np.asarray(w_attn, dtype=np.float32)
    b_attn = np.asarray(b_attn, dtype=np.float32)
    w_proj = np.asarray(w_proj, dtype=np.float32)
    b_proj = np.asarray(b_proj, dtype=np.float32)

    in_maps = make_in_maps(x, w_attn, b_attn, w_proj)
    nc = build_nc()
    res = run_bass_kernel_spmd(
        nc,
        in_maps,
        core_ids=list(range(NCORES)),
        trace=TRACE,
        **TRACE_KWARGS,
    )
    LAST_RESULT = res
    y = np.empty((B, T, C), np.float32)
    bp = b_proj.astype(np.float64)
    for b in range(B):
        y[b] = (
            res.results[2 * b]["out"].astype(np.float64)
            + res.results[2 * b + 1]["out"].astype(np.float64)
            + bp
        ).astype(np.float32)
    return y


# revision 34
# speedup vs baseline: 1.0487x; 1.0487x over previous
"""Causal self-attention (B=4, T=2048, C=1024, H=16) on 8 TRN2 NeuronCores.

Sharding: batch x head-group. Core c owns batch b=c//2 and heads
8*(c%2)..8*(c%2)+7 (4 head-pairs). Host sums the two partial [T, C]
outputs per batch and adds b_proj.

Schedule: the ScalarE exp stream (~172us) is the slow stage of the
attention inner loop, so all non-attention matmuls (qkv projections,
v production, output projection) are chopped into single-matmul
"filler" steps and drained into the PE queue inside every kc2 step.
Groups run g-major (g outer, hp inner) so the output projection of
group g unlocks early and spreads across the run. The AV pass trails
the exp by a full kc2 step (depth-2 pipeline) so it never stalls on
ACT. Diagonal chunks are triangularly tightened (fully-masked 128-col
blocks are skipped in scores/AV/exp) and the remaining per-chunk
triangle is masked additively (-1e5) in the scores PSUM before exp,
so the strict-FIFO ACT exp is the only downstream consumer.

Per core (all matmuls bf16, fp32 PSUM):
  - q,k projections dim-major (qT/kT [128, T], head-pair h0/h1 on
    partition halves); v produced key-major via x-stationary matmuls
    (no PE transposes), all 4 head-pairs per pass (N=512 moving),
    evacuated (+bias) straight into the AV operand layout.
  - Scores per head-pair: sT = kT.T @ qT with the two heads on PE row
    groups 0-63/64-127 (row-tiled, streams concurrently).
  - AV is p-stationary: per 128-query block the exp'd scores load as
    weights (128 cols, fast weight load) and only the 65 v+ones
    columns stream, halving AV moving columns; outputs land
    query-major [q, qb, d] with the ones-column Z sums at d=64.
    PSUM gotcha: matmul start=True clears has_written for the whole
    bank, so only the first matmul into each AV tile carries it.
  - Normalize is a per-partition reciprocal + multiply (no partition
    shifts); one tiled transpose-DMA per (hp, g) returns y to
    dim-major yu[d, hp, q] (dma_start_transpose tiles columns by 128:
    out[dp, a, q] = in[q, 128a + dp]).
  - Projection contracts all 128 dims of each head-pair at once
    (K=128), accumulating the 4 head-pairs into one PSUM tile.
"""

import numpy as np

import concourse.bass as bass
import concourse.mybir as mybir
import concourse.tile as tile
from concourse import bacc
from concourse.bass_utils import run_bass_kernel_spmd

B, T, C, H = 4, 2048, 1024, 16
NCORES = 8
HP = 4                      # head-pairs per core
QG = T // 512               # 4 query groups
CPB = T // 128              # 16 key chunks
f32 = mybir.dt.float32
bf16 = mybir.dt.bfloat16
EXP = mybir.ActivationFunctionType.Exp

TRACE = False
DEBUG_YU = False
TRACE_KWARGS = {}
LAST_RESULT = None
_NC_CACHE = None


def _emit(tc, x16, w16, wv16, b_qk, bv, wp16, out):
    nc = tc.nc
    with (
        tc.tile_pool(name="const", bufs=1) as constp,
        tc.tile_pool(name="persist", bufs=1) as persp,
        tc.tile_pool(name="stage", bufs=4) as stp,
        tc.tile_pool(name="pt", bufs=4) as ptp,
        tc.tile_pool(name="ost", bufs=4) as ostp,
        tc.tile_pool(name="ps_s", bufs=2, space="PSUM") as ps_s,
        tc.tile_pool(name="ps_o", bufs=2, space="PSUM") as ps_o,
        tc.tile_pool(name="aux", bufs=2, space="PSUM") as auxp,
    ):
        bqksb = constp.tile([128, 2 * HP], f32)
        bvsb = constp.tile([1, 512], f32)
        x16sb = constp.tile([128, 8, T], bf16)
        w16sb = constp.tile([128, 8, 1024], bf16)
        wv16sb = constp.tile([128, 8, 512], bf16)
        # interleave per contraction chunk: first qk + v matmuls can
        # issue as soon as the first chunks land (DMA issue itself is
        # ~0.6us per descriptor, so order = dependency order)
        for kc in range(8):
            nc.sync.dma_start(w16sb[:, kc, :], w16[:, kc, :])
            nc.sync.dma_start(x16sb[:, kc, 0:1024], x16[:, kc, 0:1024])
            nc.sync.dma_start(wv16sb[:, kc, :], wv16[:, kc, :])
            if kc == 1:
                nc.sync.dma_start(bqksb[:], b_qk)
                nc.sync.dma_start(bvsb[:], bv)
        for kc in range(8):
            nc.sync.dma_start(x16sb[:, kc, 1024:2048], x16[:, kc, 1024:2048])
        wp16sb = constp.tile([128, HP, C], bf16)
        nc.sync.dma_start(wp16sb[:], wp16)
        c1 = constp.tile([128, CPB], bf16)
        nc.gpsimd.memset(c1[:], 1.0)
        # v bias broadcast over key partitions: bvb[:, hp, :] = b_v[hp dims]
        bvb = constp.tile([128, HP, 128], f32)
        for hp in range(HP):
            nc.gpsimd.partition_broadcast(
                bvb[:, hp, :], bvsb[0:1, 128 * hp : 128 * (hp + 1)], channels=128
            )

        # additive causal triangle, applied to scores PSUM before exp:
        # 0 where col >= partition (keep), -1e5 where masked
        negtri = constp.tile([128, 128], f32)
        nc.gpsimd.memset(negtri[:], 0.0)
        nc.gpsimd.affine_select(
            out=negtri[:], in_=negtri[:], compare_op=mybir.AluOpType.is_ge,
            fill=-1.0e5, base=0, channel_multiplier=-1, pattern=[[1, 128]],
        )

        qT = persp.tile([128, HP, T], bf16, name="qT")
        kT = persp.tile([128, HP, T], bf16, name="kT")
        vaug = persp.tile([128, HP, CPB, 130], bf16, name="vaug")
        yu = persp.tile([128, HP, T], bf16, name="yu")
        _DBG = {}
        if DEBUG_YU:
            _DBG["zdbg"] = persp.tile([128, HP, QG, 2, 4], f32, name="zdbg")

        for hp in range(HP):
            nc.vector.tensor_copy(vaug[:, hp, :, 64:65], c1[:].unsqueeze(2))
            nc.vector.tensor_copy(vaug[:, hp, :, 129:130], c1[:].unsqueeze(2))

        # ---- filler units: generators yielding (cols, emit_fn) ----

        def gen_qk(hp, np_):
            for t in range(2):  # 0=q, 1=k
                dst = qT if t == 0 else kT
                msl = slice(256 * hp + 128 * t, 256 * hp + 128 * (t + 1))
                for i in range(2):
                    n = 2 * np_ + i
                    box = {}

                    def alloc(box=box, hp=hp, np_=np_, t=t, i=i):
                        box["ps"] = auxp.tile(
                            [128, 512], f32, name=f"qk_{hp}_{np_}_{t}_{i}", tag="aux"
                        )

                    for kc in range(8):
                        def mm(kc=kc, n=n, msl=msl, box=box, alloc=alloc):
                            if kc == 0:
                                alloc()
                            nc.tensor.matmul(
                                box["ps"][:],
                                w16sb[:, kc, msl],
                                x16sb[:, kc, 512 * n : 512 * (n + 1)],
                                start=(kc == 0),
                                stop=(kc == 7),
                            )
                        yield (512, mm)

                    def evac(box=box, dst=dst, n=n, hp=hp, t=t):
                        nc.vector.tensor_scalar_add(
                            dst[:, hp, 512 * n : 512 * (n + 1)],
                            box["ps"][:],
                            bqksb[:, 2 * hp + t : 2 * hp + t + 1],
                        )
                    yield (0, evac)

        def gen_v(kbq):
            for kb in range(4 * kbq, 4 * kbq + 4):
                box = {}

                def alloc(box=box, kb=kb):
                    box["ps"] = auxp.tile([128, 512], f32, name=f"vt_{kb}", tag="aux")

                for kc in range(8):
                    def mm(kc=kc, kb=kb, box=box, alloc=alloc):
                        if kc == 0:
                            alloc()
                        nc.tensor.matmul(
                            box["ps"][:],
                            x16sb[:, kc, 128 * kb : 128 * (kb + 1)],
                            wv16sb[:, kc, :],
                            start=(kc == 0),
                            stop=(kc == 7),
                        )
                    yield (512, mm)

                def evac(box=box, kb=kb):
                    for hp in range(HP):
                        dst = vaug[:, hp, kb, 0:130].rearrange(
                            "p (two x) -> p two x", x=65
                        )[:, :, 0:64]
                        nc.vector.tensor_tensor(
                            dst,
                            box["ps"][:, 128 * hp : 128 * (hp + 1)].rearrange(
                                "p (two d) -> p two d", d=64
                            ),
                            bvb[:, hp, :].rearrange("p (two d) -> p two d", d=64),
                            op=mybir.AluOpType.add,
                        )
                yield (0, evac)

        def gen_proj(rt):
            rsl = slice(128 * rt, 128 * (rt + 1))
            for jn in range(2):
                nsl = slice(512 * jn, 512 * (jn + 1))
                box = {}

                def alloc(box=box, rt=rt, jn=jn):
                    box["ps"] = auxp.tile(
                        [128, 512], f32, name=f"pp_{rt}_{jn}", tag="aux"
                    )

                for hp in range(HP):
                    def mm(hp=hp, rsl=rsl, nsl=nsl, box=box, alloc=alloc):
                        if hp == 0:
                            alloc()
                        nc.tensor.matmul(
                            box["ps"][:],
                            yu[:, hp, rsl],
                            wp16sb[:, hp, nsl],
                            start=(hp == 0),
                            stop=(hp == HP - 1),
                        )
                    yield (512, mm)

                def evac(box=box, rt=rt, jn=jn, rsl=rsl, nsl=nsl):
                    ost = ostp.tile([128, 512], f32, name=f"ost_{rt}_{jn}", tag="ost")
                    nc.vector.tensor_copy(ost[:], box["ps"][:])
                    nc.sync.dma_start(out[rsl, nsl], ost[:])
                yield (0, evac)

        # dependency-ordered unit queue; proj units are appended as
        # their inputs (yu rows for group g, all head-pairs) complete
        units = [
            ("qk00", gen_qk(0, 0)), ("v0", gen_v(0)),
            ("qk10", gen_qk(1, 0)), ("qk20", gen_qk(2, 0)), ("qk30", gen_qk(3, 0)),
            ("v1", gen_v(1)),
            ("qk01", gen_qk(0, 1)), ("qk11", gen_qk(1, 1)),
            ("v2", gen_v(2)),
            ("qk21", gen_qk(2, 1)), ("qk31", gen_qk(3, 1)),
            ("v3", gen_v(3)),
        ]
        done = set()
        cursor = [0]

        def fill(budget):
            while cursor[0] < len(units):
                name, g_ = units[cursor[0]]
                advanced = False
                for cols, fn in g_:
                    fn()
                    advanced = True
                    budget -= cols
                    if budget <= 0:
                        return
                if not advanced or True:
                    done.add(name)
                    cursor[0] += 1

        def require(name):
            while name not in done:
                assert cursor[0] < len(units), f"unit {name} not queued"
                uname, g_ = units[cursor[0]]
                for cols, fn in g_:
                    fn()
                done.add(uname)
                cursor[0] += 1

        # ---- attention ----

        def att(hp, g):
            nkc2 = 2 * g + 2
            gsl = slice(512 * g, 512 * (g + 1))
            # AV output is query-major: ots[h][q, qb, d] with the Z
            # (ones-column) sums at d=64, so normalization is a cheap
            # per-partition scalar multiply (no partition shifts)
            # padded to 128 so each query-block lands bank-aligned
            ots = [
                ps_o.tile([128, 4, 128], f32, name=f"ot_{hp}_{g}_{h}", tag="ot")
                for h in range(2)
            ]

            def off_of(kc):
                j = kc - 4 * g
                return 128 * j if j > 0 else 0

            def scores(kc2):
                sps = [
                    ps_s.tile([128, 1024], f32, name=f"sp_{hp}_{g}_{kc2}_{h}", tag="sp")
                    for h in range(2)
                ]
                for half in range(2):
                    kc = 2 * kc2 + half
                    off = off_of(kc)
                    for h in range(2):
                        hsl = slice(64 * h, 64 * h + 64)
                        nc.tensor.matmul(
                            sps[h][:, 512 * half + off : 512 * (half + 1)],
                            kT[hsl, hp, 128 * kc : 128 * (kc + 1)],
                            qT[hsl, hp, 512 * g + off : 512 * (g + 1)],
                            start=True,
                            stop=True,
                        )
                return sps

            def expmask(kc2, sps):
                # mask diagonal triangles additively in PSUM before exp:
                # the only consumer afterwards is the strict-FIFO ACT
                # exp, so no engine can observe unmasked scores
                for half in range(2):
                    kc = 2 * kc2 + half
                    j = kc - 4 * g
                    if j < 0:
                        continue
                    for h in range(2):
                        v_ = sps[h][:, 512 * half + 128 * j : 512 * half + 128 * (j + 1)]
                        nc.vector.tensor_tensor(
                            v_, v_, negtri[:], op=mybir.AluOpType.add
                        )
                pt = ptp.tile([128, 2, 1024], bf16, name=f"pt_{hp}_{g}_{kc2}", tag="pt")
                s0 = 512 * 0 + off_of(2 * kc2)  # exp span start (skip all-masked)
                for h in range(2):
                    nc.scalar.activation(
                        pt[:, h, s0:1024], sps[h][:, s0:1024], EXP, scale=0.125
                    )
                return pt

            def av(kc2, pt):
                # p-stationary: per 128-query block, load exp-scores as
                # weights (128 cols, FWL) and stream only the 65 v+ones
                # columns -- ~2x fewer moving columns than p-moving
                for h in range(2):
                    for half in range(2):
                        kc = 2 * kc2 + half
                        j = kc - 4 * g
                        for qb in range(max(j, 0), 4):
                            # start=True clears has_written for the WHOLE
                            # bank, so only the very first matmul into the
                            # tile may carry it; later first-writes rely on
                            # accumulate-mode overwriting cleared elements
                            nc.tensor.matmul(
                                ots[h][:, qb, 0:65],
                                pt[:, h, 512 * half + 128 * qb : 512 * half + 128 * (qb + 1)],
                                vaug[:, hp, kc, 65 * h : 65 * h + 65],
                                start=(kc == 0 and qb == 0),
                                stop=(kc == 4 * g + qb),
                                skip_group_check=True,
                            )

            # depth-1 software pipeline with filler drain between the
            # next scores and the exp-dependent av
            # depth-2: av(k-1) issues after scores(k+1), so its exp
            # finished a full step earlier and the PE never stalls on ACT
            cur = scores(0)
            pend = None
            for kc2 in range(nkc2):
                pt = expmask(kc2, cur)
                if kc2 + 1 < nkc2:
                    cur = scores(kc2 + 1)
                if pend is not None:
                    av(*pend)
                fill(2560)
                pend = (kc2, pt)
            av(*pend)

            # yu2[q, qb, 64h+d] = ots[h][q, qb, d] / Z; one tiled
            # transpose-DMA then yields yu[64h+d, hp, 128qb+q]
            # (dma_start_transpose: out[dp, a, q] = in[q, 128a + dp])
            yu2 = stp.tile([128, 4, 128], bf16, name=f"yu2_{hp}_{g}", tag="yu2")
            for h in range(2):
                rz = stp.tile([128, 4], f32, name=f"rz_{hp}_{g}_{h}", tag="rz")
                nc.vector.reciprocal_approx_fast(
                    rz[:], ots[h][:, :, 64:65].rearrange("p a one -> p (a one)")
                )
                if DEBUG_YU:
                    zd = _DBG["zdbg"]
                    nc.vector.tensor_copy(
                        zd[:, hp, g, h, :],
                        ots[h][:, :, 64:65].rearrange("p a one -> p (a one)"),
                    )
                nc.vector.tensor_tensor(
                    yu2[:, :, 64 * h : 64 * h + 64],
                    ots[h][:, :, 0:64],
                    rz[:].unsqueeze(2).broadcast_to([128, 4, 64]),
                    op=mybir.AluOpType.mult,
                )
            nc.sync.dma_start_transpose(
                yu[:, hp, gsl].rearrange("p (a b) -> p a b", b=128),
                yu2[:],
            )

        # ---- g-major schedule ----
        # zero the two score-PSUM ring slots once: exp spans include
        # columns the tightened scores never write; on a fresh device
        # those would otherwise be uninitialized (NaN risk)
        for i in range(2):
            spz = ps_s.tile([128, 1024], f32, name=f"spz_{i}", tag="sp")
            nc.vector.memset(spz[:], 0.0)
        require("qk00")
        require("v0")
        for g in range(QG):
            require(f"v{g}")
            for hp in range(HP):
                require(f"qk{hp}{g // 2}")
                att(hp, g)
            for rt in range(4 * g, 4 * (g + 1)):
                units.append((f"proj{rt}", gen_proj(rt)))
        # drain whatever fillers remain (late proj units)
        fill(1 << 30)
        if DEBUG_YU:
            ydbg = tc.nc.dram_tensor(
                "ydbg", [128, HP, T], bf16, kind="ExternalOutput"
            ).ap()
            nc.sync.dma_start(ydbg, yu[:])
            zdbg_d = tc.nc.dram_tensor(
                "zdbg", [128, HP, QG, 2, 4], f32, kind="ExternalOutput"
            ).ap()
            nc.sync.dma_start(zdbg_d, _DBG["zdbg"][:])


def build_nc():
    global _NC_CACHE
    if _NC_CACHE is not None:
        return _NC_CACHE
    nc = bacc.Bacc("TRN2", target_bir_lowering=False, debug=False)
    x16 = nc.dram_tensor("x16", [128, 8, T], bf16, kind="ExternalInput").ap()
    w16 = nc.dram_tensor("w16", [128, 8, 1024], bf16, kind="ExternalInput").ap()
    wv16 = nc.dram_tensor("wv16", [128, 8, 512], bf16, kind="ExternalInput").ap()
    b_qk = nc.dram_tensor("b_qk", [128, 2 * HP], f32, kind="ExternalInput").ap()
    bv = nc.dram_tensor("bv", [1, 512], f32, kind="ExternalInput").ap()
    wp16 = nc.dram_tensor("wp16", [128, HP, C], bf16, kind="ExternalInput").ap()
    out = nc.dram_tensor("out", [T, C], f32, kind="ExternalOutput").ap()
    with tile.TileContext(nc) as tc:
        _emit(tc, x16, w16, wv16, b_qk, bv, wp16, out)
    nc.compile()
    _NC_CACHE = nc
    return nc


def make_in_maps(x, w_attn, b_attn, w_proj):
    import ml_dtypes

    bf16np = ml_dtypes.bfloat16
    in_maps = []
    for core in range(NCORES):
        b, hg = divmod(core, 2)
        xb = np.asarray(x[b], np.float32)  # [T, C]
        # x16[p, kc, n] = xb[n, 128*kc + p]
        x16 = np.ascontiguousarray(
            xb.T.reshape(8, 128, T).transpose(1, 0, 2).astype(bf16np)
        )
        # w16[p, kc, hp*256 + t*128 + d] = w_attn[128*kc + p, tC + 512hg + 128hp + d]
        cols = []
        vcols = []
        for hp in range(HP):
            for toff in (0, C):
                c0 = toff + 512 * hg + 128 * hp
                cols.append(w_attn[:, c0 : c0 + 128])
            c0 = 2 * C + 512 * hg + 128 * hp
            vcols.append(w_attn[:, c0 : c0 + 128])
        wt = np.concatenate(cols, axis=1)  # [1024, 1024]
        w16 = np.ascontiguousarray(
            wt.reshape(8, 128, 1024).transpose(1, 0, 2).astype(bf16np)
        )
        wv = np.concatenate(vcols, axis=1)  # [1024, 512]
        wv16 = np.ascontiguousarray(
            wv.reshape(8, 128, 512).transpose(1, 0, 2).astype(bf16np)
        )
        b_qk = np.zeros((128, 2 * HP), np.float32)
        for hp in range(HP):
            for t, toff in enumerate((0, C)):
                c0 = toff + 512 * hg + 128 * hp
                b_qk[:, 2 * hp + t] = b_attn[c0 : c0 + 128]
        bv = np.zeros((1, 512), np.float32)
        for hp in range(HP):
            c0 = 2 * C + 512 * hg + 128 * hp
            bv[0, 128 * hp : 128 * hp + 128] = b_attn[c0 : c0 + 128]
        wp16 = np.ascontiguousarray(
            w_proj[512 * hg : 512 * hg + 512, :]
            .reshape(4, 128, C)
            .transpose(1, 0, 2)
            .astype(bf16np)
        )
        in_maps.append(
            {"x16": x16, "w16": w16, "wv16": wv16, "b_qk": b_qk, "bv": bv, "wp16": wp16}
        )
    return in_maps


def kernel(x, w_attn, b_attn, w_proj, b_proj):
    global LAST_RESULT
    x = np.asarray(x, dtype=np.float32)
    w_attn = np.asarray(w_attn, dtype=np.float32)
    b_attn = np.asarray(b_attn, dtype=np.float32)
    w_proj = np.asarray(w_proj, dtype=np.float32)
    b_proj = np.asarray(b_proj, dtype=np.float32)

    in_maps = make_in_maps(x, w_attn, b_attn, w_proj)
    nc = build_nc()
    res = run_bass_kernel_spmd(
        nc,
        in_maps,
        core_ids=list(range(NCORES)),
        trace=TRACE,
        **TRACE_KWARGS,
    )
    LAST_RESULT = res
    y = np.empty((B, T, C), np.float32)
    bp = b_proj.astype(np.float64)
    for b in range(B):
        y[b] = (
            res.results[2 * b]["out"].astype(np.float64)
            + res.results[2 * b + 1]["out"].astype(np.float64)
            + bp
        ).astype(np.float32)
    return y


# revision 35
# speedup vs baseline: 1.0633x; 1.0139x over previous
"""Causal self-attention (B=4, T=2048, C=1024, H=16) on 8 TRN2 NeuronCores.

Sharding: batch x head-group. Core c owns batch b=c//2 and heads
8*(c%2)..8*(c%2)+7 (4 head-pairs). Host sums the two partial [T, C]
outputs per batch and adds b_proj.

Schedule: the ScalarE exp stream (~172us) is the slow stage of the
attention inner loop, so all non-attention matmuls (qkv projections,
v production, output projection) are chopped into single-matmul
"filler" steps and drained into the PE queue inside every kc2 step.
Groups run g-major (g outer, hp inner) so the output projection of
group g unlocks early and spreads across the run. The AV pass trails
the exp by a full kc2 step (depth-2 pipeline) so it never stalls on
ACT. Diagonal chunks are triangularly tightened (fully-masked 128-col
blocks are skipped in scores/AV/exp) and the remaining per-chunk
triangle is masked additively (-1e5) in the scores PSUM before exp,
so the strict-FIFO ACT exp is the only downstream consumer.

Per core (all matmuls bf16, fp32 PSUM):
  - q,k projections dim-major (qT/kT [128, T], head-pair h0/h1 on
    partition halves); v produced key-major via x-stationary matmuls
    (no PE transposes), all 4 head-pairs per pass (N=512 moving),
    evacuated (+bias) straight into the AV operand layout.
  - Scores per head-pair: sT = kT.T @ qT with the two heads on PE row
    groups 0-63/64-127 (row-tiled, streams concurrently).
  - AV is p-stationary: per 128-query block the exp'd scores load as
    weights (128 cols, fast weight load) and only the 65 v+ones
    columns stream, halving AV moving columns; outputs land
    query-major [q, qb, d] with the ones-column Z sums at d=64.
    PSUM gotcha: matmul start=True clears has_written for the whole
    bank, so only the first matmul into each AV tile carries it.
  - Normalize is a per-partition reciprocal + multiply (no partition
    shifts); one tiled transpose-DMA per (hp, g) returns y to
    dim-major yu[d, hp, q] (dma_start_transpose tiles columns by 128:
    out[dp, a, q] = in[q, 128a + dp]).
  - Projection contracts all 128 dims of each head-pair at once
    (K=128), accumulating the 4 head-pairs into one PSUM tile.
"""

import numpy as np

import concourse.bass as bass
import concourse.mybir as mybir
import concourse.tile as tile
from concourse import bacc
from concourse.bass_utils import run_bass_kernel_spmd

B, T, C, H = 4, 2048, 1024, 16
NCORES = 8
HP = 4                      # head-pairs per core
QG = T // 512               # 4 query groups
CPB = T // 128              # 16 key chunks
f32 = mybir.dt.float32
bf16 = mybir.dt.bfloat16
EXP = mybir.ActivationFunctionType.Exp

TRACE = False
DEBUG_YU = False
TRACE_KWARGS = {}
LAST_RESULT = None
_NC_CACHE = None


def _emit(tc, x16, w16, wv16, b_qk, bv, wp16, out):
    nc = tc.nc
    with (
        tc.tile_pool(name="const", bufs=1) as constp,
        tc.tile_pool(name="persist", bufs=1) as persp,
        tc.tile_pool(name="stage", bufs=4) as stp,
        tc.tile_pool(name="pt", bufs=4) as ptp,
        tc.tile_pool(name="ost", bufs=4) as ostp,
        tc.tile_pool(name="ps_s", bufs=2, space="PSUM") as ps_s,
        tc.tile_pool(name="ps_o", bufs=2, space="PSUM") as ps_o,
        tc.tile_pool(name="aux", bufs=2, space="PSUM") as auxp,
    ):
        bqksb = constp.tile([128, 2 * HP], f32)
        bvsb = constp.tile([1, 512], f32)
        x16sb = constp.tile([128, 8, T], bf16)
        w16sb = constp.tile([128, 8, 1024], bf16)
        wv16sb = constp.tile([128, 8, 512], bf16)
        # interleave per contraction chunk: first qk + v matmuls can
        # issue as soon as the first chunks land (DMA issue itself is
        # ~0.6us per descriptor, so order = dependency order)
        # w16/x16 pairs first: DMA issue is ~0.65us per descriptor and
        # the first qk matmuls gate on these; wv16 (used by the v unit,
        # which runs after qk) and the biases issue afterwards
        for kc in range(8):
            nc.sync.dma_start(w16sb[:, kc, :], w16[:, kc, :])
            nc.sync.dma_start(x16sb[:, kc, 0:1024], x16[:, kc, 0:1024])
        for kc in range(8):
            nc.sync.dma_start(wv16sb[:, kc, :], wv16[:, kc, :])
            if kc == 1:
                nc.sync.dma_start(bqksb[:], b_qk)
                nc.sync.dma_start(bvsb[:], bv)
        for kc in range(8):
            nc.sync.dma_start(x16sb[:, kc, 1024:2048], x16[:, kc, 1024:2048])
        wp16sb = constp.tile([128, HP, C], bf16)
        nc.sync.dma_start(wp16sb[:], wp16)
        c1 = constp.tile([128, CPB], bf16)
        nc.gpsimd.memset(c1[:], 1.0)
        # v bias broadcast over key partitions: bvb[:, hp, :] = b_v[hp dims]
        bvb = constp.tile([128, HP, 128], f32)
        for hp in range(HP):
            nc.gpsimd.partition_broadcast(
                bvb[:, hp, :], bvsb[0:1, 128 * hp : 128 * (hp + 1)], channels=128
            )

        # additive causal triangle, applied to scores PSUM before exp:
        # 0 where col >= partition (keep), -1e5 where masked
        negtri = constp.tile([128, 128], f32)
        nc.gpsimd.memset(negtri[:], 0.0)
        nc.gpsimd.affine_select(
            out=negtri[:], in_=negtri[:], compare_op=mybir.AluOpType.is_ge,
            fill=-1.0e5, base=0, channel_multiplier=-1, pattern=[[1, 128]],
        )

        qT = persp.tile([128, HP, T], bf16, name="qT")
        kT = persp.tile([128, HP, T], bf16, name="kT")
        vaug = persp.tile([128, HP, CPB, 130], bf16, name="vaug")
        yu = persp.tile([128, HP, T], bf16, name="yu")
        _DBG = {}
        if DEBUG_YU:
            _DBG["zdbg"] = persp.tile([128, HP, QG, 2, 4], f32, name="zdbg")

        for hp in range(HP):
            nc.vector.tensor_copy(vaug[:, hp, :, 64:65], c1[:].unsqueeze(2))
            nc.vector.tensor_copy(vaug[:, hp, :, 129:130], c1[:].unsqueeze(2))

        # ---- filler units: generators yielding (cols, emit_fn) ----

        def gen_qk(hp, np_):
            for t in range(2):  # 0=q, 1=k
                dst = qT if t == 0 else kT
                msl = slice(256 * hp + 128 * t, 256 * hp + 128 * (t + 1))
                for i in range(2):
                    n = 2 * np_ + i
                    box = {}

                    def alloc(box=box, hp=hp, np_=np_, t=t, i=i):
                        box["ps"] = auxp.tile(
                            [128, 512], f32, name=f"qk_{hp}_{np_}_{t}_{i}", tag="aux"
                        )

                    for kc in range(8):
                        def mm(kc=kc, n=n, msl=msl, box=box, alloc=alloc):
                            if kc == 0:
                                alloc()
                            nc.tensor.matmul(
                                box["ps"][:],
                                w16sb[:, kc, msl],
                                x16sb[:, kc, 512 * n : 512 * (n + 1)],
                                start=(kc == 0),
                                stop=(kc == 7),
                            )
                        yield (512, mm)

                    def evac(box=box, dst=dst, n=n, hp=hp, t=t):
                        nc.vector.tensor_scalar_add(
                            dst[:, hp, 512 * n : 512 * (n + 1)],
                            box["ps"][:],
                            bqksb[:, 2 * hp + t : 2 * hp + t + 1],
                        )
                    yield (0, evac)

        def gen_v(kbq):
            for kb in range(4 * kbq, 4 * kbq + 4):
                box = {}

                def alloc(box=box, kb=kb):
                    box["ps"] = auxp.tile([128, 512], f32, name=f"vt_{kb}", tag="aux")

                for kc in range(8):
                    def mm(kc=kc, kb=kb, box=box, alloc=alloc):
                        if kc == 0:
                            alloc()
                        nc.tensor.matmul(
                            box["ps"][:],
                            x16sb[:, kc, 128 * kb : 128 * (kb + 1)],
                            wv16sb[:, kc, :],
                            start=(kc == 0),
                            stop=(kc == 7),
                        )
                    yield (512, mm)

                def evac(box=box, kb=kb):
                    for hp in range(HP):
                        dst = vaug[:, hp, kb, 0:130].rearrange(
                            "p (two x) -> p two x", x=65
                        )[:, :, 0:64]
                        nc.vector.tensor_tensor(
                            dst,
                            box["ps"][:, 128 * hp : 128 * (hp + 1)].rearrange(
                                "p (two d) -> p two d", d=64
                            ),
                            bvb[:, hp, :].rearrange("p (two d) -> p two d", d=64),
                            op=mybir.AluOpType.add,
                        )
                yield (0, evac)

        def gen_proj(rt):
            rsl = slice(128 * rt, 128 * (rt + 1))
            for jn in range(2):
                nsl = slice(512 * jn, 512 * (jn + 1))
                box = {}

                def alloc(box=box, rt=rt, jn=jn):
                    box["ps"] = auxp.tile(
                        [128, 512], f32, name=f"pp_{rt}_{jn}", tag="aux"
                    )

                for hp in range(HP):
                    def mm(hp=hp, rsl=rsl, nsl=nsl, box=box, alloc=alloc):
                        if hp == 0:
                            alloc()
                        nc.tensor.matmul(
                            box["ps"][:],
                            yu[:, hp, rsl],
                            wp16sb[:, hp, nsl],
                            start=(hp == 0),
                            stop=(hp == HP - 1),
                        )
                    yield (512, mm)

                def evac(box=box, rt=rt, jn=jn, rsl=rsl, nsl=nsl):
                    ost = ostp.tile([128, 512], f32, name=f"ost_{rt}_{jn}", tag="ost")
                    nc.vector.tensor_copy(ost[:], box["ps"][:])
                    nc.sync.dma_start(out[rsl, nsl], ost[:])
                yield (0, evac)

        # dependency-ordered unit queue; proj units are appended as
        # their inputs (yu rows for group g, all head-pairs) complete
        units = [
            ("qk00", gen_qk(0, 0)), ("v0", gen_v(0)),
            ("qk10", gen_qk(1, 0)), ("qk20", gen_qk(2, 0)), ("qk30", gen_qk(3, 0)),
            ("v1", gen_v(1)),
            ("qk01", gen_qk(0, 1)), ("qk11", gen_qk(1, 1)),
            ("v2", gen_v(2)),
            ("qk21", gen_qk(2, 1)), ("qk31", gen_qk(3, 1)),
            ("v3", gen_v(3)),
        ]
        done = set()
        cursor = [0]

        def fill(budget):
            while cursor[0] < len(units):
                name, g_ = units[cursor[0]]
                advanced = False
                for cols, fn in g_:
                    fn()
                    advanced = True
                    budget -= cols
                    if budget <= 0:
                        return
                if not advanced or True:
                    done.add(name)
                    cursor[0] += 1

        def require(name):
            while name not in done:
                assert cursor[0] < len(units), f"unit {name} not queued"
                uname, g_ = units[cursor[0]]
                for cols, fn in g_:
                    fn()
                done.add(uname)
                cursor[0] += 1

        # ---- attention ----

        def att(hp, g):
            nkc2 = 2 * g + 2
            gsl = slice(512 * g, 512 * (g + 1))
            # AV output is query-major: ots[h][q, qb, d] with the Z
            # (ones-column) sums at d=64, so normalization is a cheap
            # per-partition scalar multiply (no partition shifts)
            # padded to 128 so each query-block lands bank-aligned
            ots = [
                ps_o.tile([128, 4, 128], f32, name=f"ot_{hp}_{g}_{h}", tag="ot")
                for h in range(2)
            ]

            def off_of(kc):
                j = kc - 4 * g
                return 128 * j if j > 0 else 0

            def scores(kc2):
                sps = [
                    ps_s.tile([128, 1024], f32, name=f"sp_{hp}_{g}_{kc2}_{h}", tag="sp")
                    for h in range(2)
                ]
                for half in range(2):
                    kc = 2 * kc2 + half
                    off = off_of(kc)
                    for h in range(2):
                        hsl = slice(64 * h, 64 * h + 64)
                        nc.tensor.matmul(
                            sps[h][:, 512 * half + off : 512 * (half + 1)],
                            kT[hsl, hp, 128 * kc : 128 * (kc + 1)],
                            qT[hsl, hp, 512 * g + off : 512 * (g + 1)],
                            start=True,
                            stop=True,
                        )
                return sps

            def expmask(kc2, sps):
                # mask diagonal triangles additively in PSUM before exp:
                # the only consumer afterwards is the strict-FIFO ACT
                # exp, so no engine can observe unmasked scores
                for half in range(2):
                    kc = 2 * kc2 + half
                    j = kc - 4 * g
                    if j < 0:
                        continue
                    for h in range(2):
                        v_ = sps[h][:, 512 * half + 128 * j : 512 * half + 128 * (j + 1)]
                        nc.vector.tensor_tensor(
                            v_, v_, negtri[:], op=mybir.AluOpType.add
                        )
                pt = ptp.tile([128, 2, 1024], bf16, name=f"pt_{hp}_{g}_{kc2}", tag="pt")
                s0 = 512 * 0 + off_of(2 * kc2)  # exp span start (skip all-masked)
                for h in range(2):
                    nc.scalar.activation(
                        pt[:, h, s0:1024], sps[h][:, s0:1024], EXP, scale=0.125
                    )
                return pt

            def av(kc2, pt):
                # p-stationary: per 128-query block, load exp-scores as
                # weights (128 cols, FWL) and stream only the 65 v+ones
                # columns -- ~2x fewer moving columns than p-moving
                for h in range(2):
                    for half in range(2):
                        kc = 2 * kc2 + half
                        j = kc - 4 * g
                        for qb in range(max(j, 0), 4):
                            # start=True clears has_written for the WHOLE
                            # bank, so only the very first matmul into the
                            # tile may carry it; later first-writes rely on
                            # accumulate-mode overwriting cleared elements
                            nc.tensor.matmul(
                                ots[h][:, qb, 0:65],
                                pt[:, h, 512 * half + 128 * qb : 512 * half + 128 * (qb + 1)],
                                vaug[:, hp, kc, 65 * h : 65 * h + 65],
                                start=(kc == 0 and qb == 0),
                                stop=(kc == 4 * g + qb),
                                skip_group_check=True,
                            )

            # depth-1 software pipeline with filler drain between the
            # next scores and the exp-dependent av
            # depth-2: av(k-1) issues after scores(k+1), so its exp
            # finished a full step earlier and the PE never stalls on ACT
            cur = scores(0)
            pend = None
            for kc2 in range(nkc2):
                pt = expmask(kc2, cur)
                if kc2 + 1 < nkc2:
                    cur = scores(kc2 + 1)
                if pend is not None:
                    av(*pend)
                fill(2560)
                pend = (kc2, pt)
            av(*pend)

            # yu2[q, qb, 64h+d] = ots[h][q, qb, d] / Z; one tiled
            # transpose-DMA then yields yu[64h+d, hp, 128qb+q]
            # (dma_start_transpose: out[dp, a, q] = in[q, 128a + dp])
            yu2 = stp.tile([128, 4, 128], bf16, name=f"yu2_{hp}_{g}", tag="yu2")
            for h in range(2):
                rz = stp.tile([128, 4], f32, name=f"rz_{hp}_{g}_{h}", tag="rz")
                nc.vector.reciprocal_approx_fast(
                    rz[:], ots[h][:, :, 64:65].rearrange("p a one -> p (a one)")
                )
                if DEBUG_YU:
                    zd = _DBG["zdbg"]
                    nc.vector.tensor_copy(
                        zd[:, hp, g, h, :],
                        ots[h][:, :, 64:65].rearrange("p a one -> p (a one)"),
                    )
                nc.vector.tensor_tensor(
                    yu2[:, :, 64 * h : 64 * h + 64],
                    ots[h][:, :, 0:64],
                    rz[:].unsqueeze(2).broadcast_to([128, 4, 64]),
                    op=mybir.AluOpType.mult,
                )
            nc.sync.dma_start_transpose(
                yu[:, hp, gsl].rearrange("p (a b) -> p a b", b=128),
                yu2[:],
            )

        # ---- g-major schedule ----
        # zero the two score-PSUM ring slots once: exp spans include
        # columns the tightened scores never write; on a fresh device
        # those would otherwise be uninitialized (NaN risk)
        for i in range(2):
            spz = ps_s.tile([128, 1024], f32, name=f"spz_{i}", tag="sp")
            nc.vector.memset(spz[:], 0.0)
        require("qk00")
        require("v0")
        for g in range(QG):
            require(f"v{g}")
            for hp in range(HP):
                require(f"qk{hp}{g // 2}")
                att(hp, g)
            for rt in range(4 * g, 4 * (g + 1)):
                units.append((f"proj{rt}", gen_proj(rt)))
        # drain whatever fillers remain (late proj units)
        fill(1 << 30)
        if DEBUG_YU:
            ydbg = tc.nc.dram_tensor(
                "ydbg", [128, HP, T], bf16, kind="ExternalOutput"
            ).ap()
            nc.sync.dma_start(ydbg, yu[:])
            zdbg_d = tc.nc.dram_tensor(
                "zdbg", [128, HP, QG, 2, 4], f32, kind="ExternalOutput"
            ).ap()
            nc.sync.dma_start(zdbg_d, _DBG["zdbg"][:])


def build_nc():
    global _NC_CACHE
    if _NC_CACHE is not None:
        return _NC_CACHE
    nc = bacc.Bacc("TRN2", target_bir_lowering=False, debug=False)
    x16 = nc.dram_tensor("x16", [128, 8, T], bf16, kind="ExternalInput").ap()
    w16 = nc.dram_tensor("w16", [128, 8, 1024], bf16, kind="ExternalInput").ap()
    wv16 = nc.dram_tensor("wv16", [128, 8, 512], bf16, kind="ExternalInput").ap()
    b_qk = nc.dram_tensor("b_qk", [128, 2 * HP], f32, kind="ExternalInput").ap()
    bv = nc.dram_tensor("bv", [1, 512], f32, kind="ExternalInput").ap()
    wp16 = nc.dram_tensor("wp16", [128, HP, C], bf16, kind="ExternalInput").ap()
    out = nc.dram_tensor("out", [T, C], f32, kind="ExternalOutput").ap()
    with tile.TileContext(nc) as tc:
        _emit(tc, x16, w16, wv16, b_qk, bv, wp16, out)
    nc.compile()
    _NC_CACHE = nc
    return nc


def make_in_maps(x, w_attn, b_attn, w_proj):
    import ml_dtypes

    bf16np = ml_dtypes.bfloat16
    in_maps = []
    for core in range(NCORES):
        b, hg = divmod(core, 2)
        xb = np.asarray(x[b], np.float32)  # [T, C]
        # x16[p, kc, n] = xb[n, 128*kc + p]
        x16 = np.ascontiguousarray(
            xb.T.reshape(8, 128, T).transpose(1, 0, 2).astype(bf16np)
        )
        # w16[p, kc, hp*256 + t*128 + d] = w_attn[128*kc + p, tC + 512hg + 128hp + d]
        cols = []
        vcols = []
        for hp in range(HP):
            for toff in (0, C):
                c0 = toff + 512 * hg + 128 * hp
                cols.append(w_attn[:, c0 : c0 + 128])
            c0 = 2 * C + 512 * hg + 128 * hp
            vcols.append(w_attn[:, c0 : c0 + 128])
        wt = np.concatenate(cols, axis=1)  # [1024, 1024]
        w16 = np.ascontiguousarray(
            wt.reshape(8, 128, 1024).transpose(1, 0, 2).astype(bf16np)
        )
        wv = np.concatenate(vcols, axis=1)  # [1024, 512]
        wv16 = np.ascontiguousarray(
            wv.reshape(8, 128, 512).transpose(1, 0, 2).astype(bf16np)
        )
        b_qk = np.zeros((128, 2 * HP), np.float32)
        for hp in range(HP):
            for t, toff in enumerate((0, C)):
                c0 = toff + 512 * hg + 128 * hp
                b_qk[:, 2 * hp + t] = b_attn[c0 : c0 + 128]
        bv = np.zeros((1, 512), np.float32)
        for hp in range(HP):
            c0 = 2 * C + 512 * hg + 128 * hp
            bv[0, 128 * hp : 128 * hp + 128] = b_attn[c0 : c0 + 128]
        wp16 = np.ascontiguousarray(
            w_proj[512 * hg : 512 * hg + 512, :]
            .reshape(4, 128, C)
            .transpose(1, 0, 2)
            .astype(bf16np)
        )
        in_maps.append(
            {"x16": x16, "w16": w16, "wv16": wv16, "b_qk": b_qk, "bv": bv, "wp16": wp16}
        )
    return in_maps


def kernel(x, w_attn, b_attn, w_proj, b_proj):
    global LAST_RESULT
    x = np.asarray(x, dtype=np.float32)
    w_attn = np.asarray(w_attn, dtype=np.float32)
    b_attn = np.asarray(b_attn, dtype=np.float32)
    w_proj = np.asarray(w_proj, dtype=np.float32)
    b_proj = np.asarray(b_proj, dtype=np.float32)

    in_maps = make_in_maps(x, w_attn, b_attn, w_proj)
    nc = build_nc()
    res = run_bass_kernel_spmd(
        nc,
        in_maps,
        core_ids=list(range(NCORES)),
        trace=TRACE,
        **TRACE_KWARGS,
    )
    LAST_RESULT = res
    y = np.empty((B, T, C), np.float32)
    bp = b_proj.astype(np.float64)
    for b in range(B):
        y[b] = (
            res.results[2 * b]["out"].astype(np.float64)
            + res.results[2 * b + 1]["out"].astype(np.float64)
            + bp
        ).astype(np.float32)
    return y
